# revision 6
# baseline (speedup 1.0000x reference)
"""Bass/Trainium2 kernel for nn_AdaptiveMoELayer (B=4, S=2048, D=1024, F=4096, E=4).

Strategy: data-parallel over tokens across 8 NeuronCores (each core gets
1024 tokens and all expert weights, streamed from HBM in bf16).  Every token
needs every expert (the reference computes the dense all-expert MLP and then
mixes with per-token coefficients), so there is no cross-core communication.

Per-token mixing weights: out[t] = sum_e c_e(t) * (relu(x W1_e + b1_e) W2_e
+ b2_e) with c_e(t) = u/i * [ceil(4u) >= i], i = ((e - s) mod 4) + 1, s the
sequence position (s mod 4 == t mod 4 for every 1024-token shard).  The
uncertainty head u = sigmoid(x @ Wu + bu) is computed in fp32 on-device; the
expert MLP matmuls run in bf16 with fp32 PSUM accumulation.

Compute layout per core (tokens t: 1024, split into two 512 chunks):
  h^T[f, t] = W1_e[d, f].T @ x^T[d, t]        (W1 tile stationary)
  g = bf16(relu(h^T + b1) * c_e(t))           (ACT relu+bias, DVE scale)
  out^T[d, t] += W2_e[f, d].T @ g[f, t]       (W2 tile stationary)
  out^T[d, t] += b2^T[d, e] @ C[e, t]         (tiny K=4 matmul, e=0 group)
Host transposes out^T back and stitches shards.
"""

import numpy as np
import ml_dtypes

B, S, D, F, E = 4, 2048, 1024, 4096, 4
NCORES = 8
T = B * S
TC = T // NCORES          # tokens per core
NDT = D // 128            # 8  d-tiles
NFT = F // 128            # 32 f-tiles
TCH = 512                 # token chunk (one PSUM bank of fp32)
NCH = TC // TCH           # 2

_bf16 = ml_dtypes.bfloat16
_compiled = None


def _build():
    import concourse.bass as bass
    import concourse.tile as tile
    from concourse import bacc, mybir

    f32 = mybir.dt.float32
    bf16 = mybir.dt.bfloat16
    Alu = mybir.AluOpType
    Act = mybir.ActivationFunctionType

    nc = bacc.Bacc("TRN2", target_bir_lowering=False, debug=False,
                   num_devices=NCORES)

    xtb_d = nc.dram_tensor("xtb", [D, TC], bf16, kind="ExternalInput").ap()
    xtf_d = nc.dram_tensor("xtf", [D, TC], f32, kind="ExternalInput").ap()
    w1_d = nc.dram_tensor("w1t", [E, NFT, 128, D], bf16, kind="ExternalInput").ap()
    w2_d = nc.dram_tensor("w2t", [E, NDT, 128, F], bf16, kind="ExternalInput").ap()
    b1_d = nc.dram_tensor("b1s", [128, E * NFT], f32, kind="ExternalInput").ap()
    b2_d = nc.dram_tensor("b2s", [E, D], f32, kind="ExternalInput").ap()
    wu_d = nc.dram_tensor("wus", [128, NDT], f32, kind="ExternalInput").ap()
    bu_d = nc.dram_tensor("bus", [1, 1], f32, kind="ExternalInput").ap()
    im1_d = nc.dram_tensor("im1", [E, TC], f32, kind="ExternalInput").ap()
    iinv_d = nc.dram_tensor("iinv", [E, TC], f32, kind="ExternalInput").ap()
    ones_d = nc.dram_tensor("ones", [1, 128], f32, kind="ExternalInput").ap()
    out_d = nc.dram_tensor("out", [D, TC], f32, kind="ExternalOutput").ap()

    xtb_v = xtb_d.rearrange("(dt p) t -> p dt t", p=128)
    xtf_v = xtf_d.rearrange("(dt p) t -> p dt t", p=128)
    out_v = out_d.rearrange("(dt p) t -> p dt t", p=128)

    with tile.TileContext(nc) as tc:
        with (
            tc.tile_pool(name="consts", bufs=1) as consts,
            tc.tile_pool(name="xtf", bufs=2) as xtfp,
            tc.tile_pool(name="crow", bufs=2) as crowp,
            tc.tile_pool(name="w1", bufs=3) as w1p,
            tc.tile_pool(name="w2", bufs=2) as w2p,
            tc.tile_pool(name="g", bufs=1) as gp,
            tc.tile_pool(name="hr", bufs=3) as hrp,
            tc.tile_pool(name="oacc", bufs=1) as oaccp,
            tc.tile_pool(name="ps", bufs=6, space="PSUM") as ps,
            tc.tile_pool(name="pmisc", bufs=2, space="PSUM") as pmisc,
            tc.tile_pool(name="dscratch", bufs=1, space="DRAM") as dpool,
        ):
            # ---- resident constants / inputs ----
            xtb = consts.tile([128, NDT, TC], bf16)
            nc.sync.dma_start(xtb[:], xtb_v)
            b1s = consts.tile([128, E * NFT], f32)
            nc.sync.dma_start(b1s[:], b1_d)
            b2s = consts.tile([E, D], f32)
            nc.sync.dma_start(b2s[:], b2_d)
            wus = consts.tile([128, NDT], f32)
            nc.sync.dma_start(wus[:], wu_d)
            bus = consts.tile([1, 1], f32)
            nc.sync.dma_start(bus[:], bu_d)
            im1 = consts.tile([E, TC], f32)
            nc.sync.dma_start(im1[:], im1_d)
            iinv = consts.tile([E, TC], f32)
            nc.sync.dma_start(iinv[:], iinv_d)
            ones = consts.tile([1, 128], f32)
            nc.sync.dma_start(ones[:], ones_d)

            # ---- uncertainty head: u = sigmoid(x @ Wu + bu), fp32 ----
            u_sb = consts.tile([1, TC], f32)
            pu = [pmisc.tile([1, TCH], f32, tag="pm", name=f"pu{i}") for i in range(NCH)]
            for dt in range(NDT):
                for ch in range(NCH):
                    xt = xtfp.tile([128, TCH], f32, tag="xtf", name="xt")
                    nc.sync.dma_start(
                        xt[:], xtf_v[:, dt, ch * TCH : (ch + 1) * TCH]
                    )
                    nc.tensor.matmul(
                        pu[ch][:],
                        lhsT=wus[:, dt : dt + 1],
                        rhs=xt[:],
                        start=(dt == 0),
                        stop=(dt == NDT - 1),
                    )
            for ch in range(NCH):
                nc.scalar.activation(
                    u_sb[:, ch * TCH : (ch + 1) * TCH], pu[ch][:],
                    Act.Sigmoid, bias=bus[:, 0:1],
                )

            # ---- gating coefficients c_e(t) ----
            # u4[e, t] = u[t] (replicated to partitions 0..3 via DRAM hop),
            # then c4 = u4 * iinv * (4*u4 > im1)  elementwise on 4 partitions.
            uscr = dpool.tile([1, TC], f32, name="uscr")
            nc.sync.dma_start(uscr[:], u_sb[:])
            u4 = consts.tile([E, TC], f32)
            for e in range(E):
                nc.sync.dma_start(u4[e : e + 1, :], uscr[:])
            mask = consts.tile([E, TC], f32)
            nc.vector.scalar_tensor_tensor(
                mask[:], u4[:], 4.0, im1[:], Alu.mult, Alu.is_gt
            )
            c4 = consts.tile([E, TC], f32)
            nc.vector.tensor_tensor(c4[:], u4[:], iinv[:], Alu.mult)
            nc.vector.tensor_tensor(c4[:], c4[:], mask[:], Alu.mult)

            # per-expert row on partition 0 (for PE broadcast), via DRAM hop
            cscr = dpool.tile([E, TC], f32, name="cscr")
            nc.sync.dma_start(cscr[:], c4[:])

            # broadcast tiles cbc[e][p, t] = c_e(t) for the h-scaling (bf16)
            cbc = []
            for e in range(E):
                crow = crowp.tile([1, TC], f32, tag="crow", name="crow")
                nc.sync.dma_start(crow[:], cscr[e : e + 1, :])
                cb = consts.tile([128, TC], bf16, tag=f"cbc{e}", name=f"cbc{e}")
                for ch in range(NCH):
                    pcb = pmisc.tile([128, TCH], f32, tag="pm", name=f"pcb{e}_{ch}")
                    nc.tensor.matmul(
                        pcb[:],
                        lhsT=ones[:, 0:128],
                        rhs=crow[:, ch * TCH : (ch + 1) * TCH],
                        start=True,
                        stop=True,
                    )
                    nc.vector.tensor_copy(cb[:, ch * TCH : (ch + 1) * TCH], pcb[:])
                cbc.append(cb)

            # ---- main expert loop ----
            oacc = oaccp.tile([128, NDT, TC], f32)
            for e in range(E):
                g_t = gp.tile([128, NFT, TC], bf16, tag="g", name="g_t")
                for ft in range(NFT):
                    w1t = w1p.tile([128, D], bf16, tag="w1", name="w1t")
                    nc.sync.dma_start(w1t[:], w1_d[e, ft])
                    ph = [ps.tile([128, TCH], f32, tag="ps", name=f"ph{i}") for i in range(NCH)]
                    for dt in range(NDT):
                        lhs = w1t[:, dt * 128 : (dt + 1) * 128]
                        for ch in range(NCH):
                            nc.tensor.matmul(
                                ph[ch][:],
                                lhsT=lhs,
                                rhs=xtb[:, dt, ch * TCH : (ch + 1) * TCH],
                                start=(dt == 0),
                                stop=(dt == NDT - 1),
                            )
                    b1ap = b1s[:, e * NFT + ft : e * NFT + ft + 1]
                    for ch in range(NCH):
                        hr = hrp.tile([128, TCH], bf16, tag="hr", name="hr")
                        nc.scalar.activation(hr[:], ph[ch][:], Act.Relu, bias=b1ap)
                        nc.vector.tensor_tensor(
                            g_t[:, ft, ch * TCH : (ch + 1) * TCH],
                            hr[:],
                            cbc[e][:, ch * TCH : (ch + 1) * TCH],
                            Alu.mult,
                        )
                for dti in range(NDT):
                    w2t = w2p.tile([128, F], bf16, tag="w2", name="w2t")
                    nc.sync.dma_start(w2t[:], w2_d[e, dti])
                    po = [ps.tile([128, TCH], f32, tag="ps", name=f"po{i}") for i in range(NCH)]
                    if e == 0:
                        for ch in range(NCH):
                            nc.tensor.matmul(
                                po[ch][:],
                                lhsT=b2s[:, dti * 128 : (dti + 1) * 128],
                                rhs=c4[:, ch * TCH : (ch + 1) * TCH],
                                start=True,
                                stop=False,
                            )
                    for ft in range(NFT):
                        lhs = w2t[:, ft * 128 : (ft + 1) * 128]
                        for ch in range(NCH):
                            nc.tensor.matmul(
                                po[ch][:],
                                lhsT=lhs,
                                rhs=g_t[:, ft, ch * TCH : (ch + 1) * TCH],
                                start=(e != 0 and ft == 0),
                                stop=(ft == NFT - 1),
                            )
                    for ch in range(NCH):
                        dst = oacc[:, dti, ch * TCH : (ch + 1) * TCH]
                        if e == 0:
                            nc.scalar.copy(dst, po[ch][:])
                        else:
                            nc.vector.tensor_add(dst, dst, po[ch][:])

            nc.sync.dma_start(out_v, oacc[:])

    nc.compile()
    return nc


def _host_prep(x, W1, b1, W2, b2, Wu, bu):
    """Shard + retile inputs; returns per-core in_maps."""
    xf = np.ascontiguousarray(x.reshape(T, D))
    w1t = np.ascontiguousarray(
        W1.reshape(E, NDT, 128, NFT, 128).transpose(0, 3, 2, 1, 4)
    ).reshape(E, NFT, 128, D).astype(_bf16)
    w2t = np.ascontiguousarray(
        W2.reshape(E, NFT, 128, NDT, 128).transpose(0, 3, 2, 1, 4)
    ).reshape(E, NDT, 128, F).astype(_bf16)
    b1s = np.ascontiguousarray(
        b1.reshape(E, NFT, 128).transpose(2, 0, 1).reshape(128, E * NFT)
    ).astype(np.float32)
    b2s = np.ascontiguousarray(b2).astype(np.float32)
    wus = np.ascontiguousarray(Wu[:, 0].reshape(NDT, 128).T).astype(np.float32)
    bus = np.asarray(bu, dtype=np.float32).reshape(1, 1)
    t_idx = np.arange(TC)
    i_mat = ((np.arange(E)[:, None] - t_idx[None, :]) % E) + 1  # [E, TC]
    im1 = np.ascontiguousarray(i_mat - 1).astype(np.float32)
    iinv = np.ascontiguousarray(1.0 / i_mat).astype(np.float32)
    ones = np.ones((1, 128), dtype=np.float32)

    in_maps = []
    for c in range(NCORES):
        shard = xf[c * TC : (c + 1) * TC]          # [TC, D]
        xT = np.ascontiguousarray(shard.T)          # [D, TC]
        in_maps.append({
            "xtb": xT.astype(_bf16),
            "xtf": xT.astype(np.float32),
            "w1t": w1t,
            "w2t": w2t,
            "b1s": b1s,
            "b2s": b2s,
            "wus": wus,
            "bus": bus,
            "im1": im1,
            "iinv": iinv,
            "ones": ones,
        })
    return in_maps


def kernel(x, W1, b1, W2, b2, Wu, bu):
    global _compiled
    from concourse.bass_utils import run_bass_kernel_spmd

    if _compiled is None:
        _compiled = _build()
    in_maps = _host_prep(
        np.asarray(x), np.asarray(W1), np.asarray(b1), np.asarray(W2),
        np.asarray(b2), np.asarray(Wu), np.asarray(bu),
    )
    res = run_bass_kernel_spmd(_compiled, in_maps, core_ids=list(range(NCORES)))
    kernel._last_result = res
    shards = [res.results[c]["out"].T for c in range(NCORES)]  # [TC, D] each
    return np.concatenate(shards, axis=0).reshape(B, S, D).astype(np.float32)


# revision 13
# speedup vs baseline: 1.0359x; 1.0359x over previous
"""Bass/Trainium2 kernel for nn_AdaptiveMoELayer (B=4, S=2048, D=1024, F=4096, E=4).

Strategy: data-parallel over tokens across 8 NeuronCores (each core gets
1024 tokens and all expert weights, streamed from HBM in bf16).  Every token
needs every expert (the reference computes the dense all-expert MLP and then
mixes with per-token coefficients), so there is no cross-core communication.

Per-token mixing weights: out[t] = sum_e c_e(t) * (relu(x W1_e + b1_e) W2_e
+ b2_e) with c_e(t) = u/i * [ceil(4u) >= i], i = ((e - s) mod 4) + 1, s the
sequence position (s mod 4 == t mod 4 for every 1024-token shard).  The
uncertainty head u = sigmoid(x @ Wu + bu) is computed in fp32 on-device; the
expert MLP matmuls run in bf16 with fp32 PSUM accumulation.

Compute layout per core (tokens t: 1024, split into two 512 chunks):
  h^T[f, t] = W1_e[d, f].T @ x^T[d, t]        (W1 tile stationary)
  g = bf16(relu(h^T + b1) * c_e(t))           (ACT relu+bias, DVE scale)
  out^T[d, t] += W2_e[f, d].T @ g[f, t]       (W2 tile stationary)
  out^T[d, t] += b2^T[d, e] @ C[e, t]         (tiny K=4 matmul, e=0 group)
Host transposes out^T back and stitches shards.
"""

import numpy as np
import ml_dtypes

B, S, D, F, E = 4, 2048, 1024, 4096, 4
NCORES = 8
T = B * S
TC = T // NCORES          # tokens per core
NDT = D // 128            # 8  d-tiles
NFT = F // 128            # 32 f-tiles
TCH = 512                 # token chunk (one PSUM bank of fp32)
NCH = TC // TCH           # 2

_bf16 = ml_dtypes.bfloat16
_compiled = None


def _build():
    import concourse.bass as bass
    import concourse.tile as tile
    from concourse import bacc, mybir

    f32 = mybir.dt.float32
    bf16 = mybir.dt.bfloat16
    Alu = mybir.AluOpType
    Act = mybir.ActivationFunctionType

    nc = bacc.Bacc("TRN2", target_bir_lowering=False, debug=False,
                   num_devices=NCORES)

    xtb_d = nc.dram_tensor("xtb", [D, TC], bf16, kind="ExternalInput").ap()
    xtf_d = nc.dram_tensor("xtf", [D, TC], f32, kind="ExternalInput").ap()
    w1_d = nc.dram_tensor("w1t", [E, NFT, 128, D], bf16, kind="ExternalInput").ap()
    w2_d = nc.dram_tensor("w2t", [E, NDT, 128, F], bf16, kind="ExternalInput").ap()
    b1_d = nc.dram_tensor("b1s", [128, E * NFT], f32, kind="ExternalInput").ap()
    b2_d = nc.dram_tensor("b2s", [E, D], bf16, kind="ExternalInput").ap()
    wu_d = nc.dram_tensor("wus", [128, NDT], f32, kind="ExternalInput").ap()
    bu_d = nc.dram_tensor("bus", [1, 1], f32, kind="ExternalInput").ap()
    im1_d = nc.dram_tensor("im1", [E, TC], f32, kind="ExternalInput").ap()
    iinv_d = nc.dram_tensor("iinv", [E, TC], f32, kind="ExternalInput").ap()
    ones_d = nc.dram_tensor("ones", [1, 128], bf16, kind="ExternalInput").ap()
    out_d = nc.dram_tensor("out", [D, TC], f32, kind="ExternalOutput").ap()

    xtb_v = xtb_d.rearrange("(dt p) t -> p dt t", p=128)
    xtf_v = xtf_d.rearrange("(dt p) t -> p dt t", p=128)
    out_v = out_d.rearrange("(dt p) t -> p dt t", p=128)

    with tile.TileContext(nc) as tc:
        with (
            tc.tile_pool(name="consts", bufs=1) as consts,
            tc.tile_pool(name="xtf", bufs=2) as xtfp,
            tc.tile_pool(name="crow", bufs=2) as crowp,
            tc.tile_pool(name="w1", bufs=3) as w1p,
            tc.tile_pool(name="w2", bufs=2) as w2p,
            tc.tile_pool(name="g", bufs=34) as gp,
            tc.tile_pool(name="hr", bufs=3) as hrp,
            tc.tile_pool(name="oacc", bufs=1) as oaccp,
            tc.tile_pool(name="ps", bufs=6, space="PSUM") as ps,
            tc.tile_pool(name="pmisc", bufs=2, space="PSUM") as pmisc,
            tc.tile_pool(name="dscratch", bufs=1, space="DRAM") as dpool,
        ):
            # ---- resident constants / inputs ----
            wus = consts.tile([128, NDT], f32)
            nc.sync.dma_start(wus[:], wu_d)
            bus = consts.tile([1, 1], f32)
            nc.sync.dma_start(bus[:], bu_d)
            xtb = consts.tile([128, NDT, TC], bf16)
            for dt in range(NDT):
                nc.sync.dma_start(xtb[:, dt, :], xtb_v[:, dt, :])
            b1s = consts.tile([128, E * NFT], f32)
            nc.sync.dma_start(b1s[:], b1_d)
            b2s = consts.tile([E, D], bf16)
            nc.sync.dma_start(b2s[:], b2_d)
            im1 = consts.tile([E, TC], f32)
            nc.sync.dma_start(im1[:], im1_d)
            iinv = consts.tile([E, TC], f32)
            nc.sync.dma_start(iinv[:], iinv_d)
            ones = consts.tile([1, 128], bf16)
            nc.sync.dma_start(ones[:], ones_d)

            # ---- uncertainty head: u = sigmoid(x @ Wu + bu), fp32 ----
            u_sb = consts.tile([1, TC], f32)
            pu = [pmisc.tile([1, TCH], f32, tag="pm", name=f"pu{i}") for i in range(NCH)]
            for dt in range(NDT):
                for ch in range(NCH):
                    xt = xtfp.tile([128, TCH], f32, tag="xtf", name="xt")
                    nc.sync.dma_start(
                        xt[:], xtf_v[:, dt, ch * TCH : (ch + 1) * TCH]
                    )
                    nc.tensor.matmul(
                        pu[ch][:],
                        lhsT=wus[:, dt : dt + 1],
                        rhs=xt[:],
                        start=(dt == 0),
                        stop=(dt == NDT - 1),
                    )
            for ch in range(NCH):
                nc.scalar.activation(
                    u_sb[:, ch * TCH : (ch + 1) * TCH], pu[ch][:],
                    Act.Sigmoid, bias=bus[:, 0:1],
                )

            # ---- gating coefficients c_e(t) ----
            # u4[e, t] = u[t] (replicated to partitions 0..3 via DRAM hop),
            # then c4 = u4 * iinv * (4*u4 > im1)  elementwise on 4 partitions.
            uscr = dpool.tile([1, TC], f32, name="uscr")
            nc.sync.dma_start(uscr[:], u_sb[:])
            u4 = consts.tile([E, TC], f32)
            for e in range(E):
                nc.sync.dma_start(u4[e : e + 1, :], uscr[:])
            mask = consts.tile([E, TC], f32)
            nc.vector.scalar_tensor_tensor(
                mask[:], u4[:], 4.0, im1[:], Alu.mult, Alu.is_gt
            )
            c4 = consts.tile([E, TC], f32)
            nc.vector.tensor_tensor(c4[:], u4[:], iinv[:], Alu.mult)
            nc.vector.tensor_tensor(c4[:], c4[:], mask[:], Alu.mult)
            c4b = consts.tile([E, TC], bf16)
            nc.vector.tensor_copy(c4b[:], c4[:])

            # per-expert row on partition 0 (for PE broadcast), via DRAM hop
            cscr = dpool.tile([E, TC], bf16, name="cscr")
            nc.sync.dma_start(cscr[:], c4b[:])

            # broadcast tiles cbc[e][p, t] = c_e(t) for the h-scaling (bf16)
            cbc = []
            for e in range(E):
                crow = crowp.tile([1, TC], bf16, tag="crow", name="crow")
                nc.sync.dma_start(crow[:], cscr[e : e + 1, :])
                cb = consts.tile([128, TC], bf16, tag=f"cbc{e}", name=f"cbc{e}")
                for ch in range(NCH):
                    pcb = pmisc.tile([128, TCH], f32, tag="pm", name=f"pcb{e}_{ch}")
                    nc.tensor.matmul(
                        pcb[:],
                        lhsT=ones[:, 0:128],
                        rhs=crow[:, ch * TCH : (ch + 1) * TCH],
                        start=True,
                        stop=True,
                    )
                    nc.vector.tensor_copy(cb[:, ch * TCH : (ch + 1) * TCH], pcb[:])
                cbc.append(cb)

            # ---- main expert loop ----
            oacc = oaccp.tile([128, NDT, TC], f32)
            for e in range(E):
                g_tiles = []
                for ft in range(NFT):
                    w1t = w1p.tile([128, D], bf16, tag="w1", name="w1t")
                    nc.sync.dma_start(w1t[:], w1_d[e, ft])
                    g_t = gp.tile([128, TC], bf16, tag="g", name="g_t")
                    g_tiles.append(g_t)
                    ph = [ps.tile([128, TCH], f32, tag="ps", name=f"ph{i}") for i in range(NCH)]
                    for dt in range(NDT):
                        lhs = w1t[:, dt * 128 : (dt + 1) * 128]
                        for ch in range(NCH):
                            nc.tensor.matmul(
                                ph[ch][:],
                                lhsT=lhs,
                                rhs=xtb[:, dt, ch * TCH : (ch + 1) * TCH],
                                start=(dt == 0),
                                stop=(dt == NDT - 1),
                            )
                    b1ap = b1s[:, e * NFT + ft : e * NFT + ft + 1]
                    for ch in range(NCH):
                        hr = hrp.tile([128, TCH], bf16, tag="hr", name="hr")
                        nc.scalar.activation(hr[:], ph[ch][:], Act.Relu, bias=b1ap)
                        nc.vector.tensor_tensor(
                            g_t[:, ch * TCH : (ch + 1) * TCH],
                            hr[:],
                            cbc[e][:, ch * TCH : (ch + 1) * TCH],
                            Alu.mult,
                        )
                for dti in range(NDT):
                    w2t = w2p.tile([128, F], bf16, tag="w2", name="w2t")
                    nc.sync.dma_start(w2t[:], w2_d[e, dti])
                    po = [ps.tile([128, TCH], f32, tag="ps", name=f"po{i}") for i in range(NCH)]
                    if e == 0:
                        for ch in range(NCH):
                            nc.tensor.matmul(
                                po[ch][:],
                                lhsT=b2s[:, dti * 128 : (dti + 1) * 128],
                                rhs=c4b[:, ch * TCH : (ch + 1) * TCH],
                                start=True,
                                stop=False,
                            )
                    for ft in range(NFT):
                        lhs = w2t[:, ft * 128 : (ft + 1) * 128]
                        for ch in range(NCH):
                            nc.tensor.matmul(
                                po[ch][:],
                                lhsT=lhs,
                                rhs=g_tiles[ft][:, ch * TCH : (ch + 1) * TCH],
                                start=(e != 0 and ft == 0),
                                stop=(ft == NFT - 1),
                            )
                    for ch in range(NCH):
                        dst = oacc[:, dti, ch * TCH : (ch + 1) * TCH]
                        if e == 0:
                            nc.scalar.copy(dst, po[ch][:])
                        else:
                            nc.vector.tensor_add(dst, dst, po[ch][:])
                        if e == E - 1:
                            nc.sync.dma_start(
                                out_v[:, dti, ch * TCH : (ch + 1) * TCH], dst
                            )

    nc.compile()
    return nc


def _host_prep(x, W1, b1, W2, b2, Wu, bu):
    """Shard + retile inputs; returns per-core in_maps."""
    xf = np.ascontiguousarray(x.reshape(T, D))
    w1t = np.ascontiguousarray(
        W1.reshape(E, NDT, 128, NFT, 128).transpose(0, 3, 2, 1, 4)
    ).reshape(E, NFT, 128, D).astype(_bf16)
    w2t = np.ascontiguousarray(
        W2.reshape(E, NFT, 128, NDT, 128).transpose(0, 3, 2, 1, 4)
    ).reshape(E, NDT, 128, F).astype(_bf16)
    b1s = np.ascontiguousarray(
        b1.reshape(E, NFT, 128).transpose(2, 0, 1).reshape(128, E * NFT)
    ).astype(np.float32)
    b2s = np.ascontiguousarray(b2).astype(_bf16)
    wus = np.ascontiguousarray(Wu[:, 0].reshape(NDT, 128).T).astype(np.float32)
    bus = np.asarray(bu, dtype=np.float32).reshape(1, 1)
    t_idx = np.arange(TC)
    i_mat = ((np.arange(E)[:, None] - t_idx[None, :]) % E) + 1  # [E, TC]
    im1 = np.ascontiguousarray(i_mat - 1).astype(np.float32)
    iinv = np.ascontiguousarray(1.0 / i_mat).astype(np.float32)
    ones = np.ones((1, 128), dtype=_bf16)

    in_maps = []
    for c in range(NCORES):
        shard = xf[c * TC : (c + 1) * TC]          # [TC, D]
        xT = np.ascontiguousarray(shard.T)          # [D, TC]
        in_maps.append({
            "xtb": xT.astype(_bf16),
            "xtf": xT.astype(np.float32),
            "w1t": w1t,
            "w2t": w2t,
            "b1s": b1s,
            "b2s": b2s,
            "wus": wus,
            "bus": bus,
            "im1": im1,
            "iinv": iinv,
            "ones": ones,
        })
    return in_maps


def kernel(x, W1, b1, W2, b2, Wu, bu):
    global _compiled
    from concourse.bass_utils import run_bass_kernel_spmd

    if _compiled is None:
        _compiled = _build()
    in_maps = _host_prep(
        np.asarray(x), np.asarray(W1), np.asarray(b1), np.asarray(W2),
        np.asarray(b2), np.asarray(Wu), np.asarray(bu),
    )
    res = run_bass_kernel_spmd(_compiled, in_maps, core_ids=list(range(NCORES)))
    kernel._last_result = res
    shards = [res.results[c]["out"].T for c in range(NCORES)]  # [TC, D] each
    return np.concatenate(shards, axis=0).reshape(B, S, D).astype(np.float32)
